# revision 38
# baseline (speedup 1.0000x reference)
"""Trainium2 Bass kernel for hashgrid encode + MLP + SH (nn_Hash1SH).

Contract: kernel(**inputs) takes FULL unsharded inputs, returns FULL output.
Sharding: data-parallel over points (8 cores x 32768 points), tables
replicated (host-interleaved bf16 so one gather row serves both tables).

Design notes (validated on axon trn2, 2026-08-09):
- HW indirect DMA supports exactly ONE dynamic index per partition per call
  (the rest of the out AP streams contiguously from that base), so gathers
  are one [P,1]-offset call per point-column.
- To cut call count, levels 0..DENSE_L-1 use host-precomputed dense per-cell
  tables (64B row = all 8 corners' features, [f,k] layout): 1 call/column
  instead of 8. Remaining levels gather 8B hashed rows per corner.
- bf16 feature datapath; dense-level interpolation uses inner-contiguous
  bf16 APs (DVE 2x mode eligible); fused two-table MLP via block-diagonal
  bf16 matmuls over 2-column groups; SH + final 3x3 chain point-major.
- floor(pos) is computed via convert + compare-fixup so CoreSim (truncating
  f32->i32) and HW (rounding) agree exactly.
- kernel() runs a cached jit fast path with device-resident tables;
  BASS_TRACE=1 switches to run_bass_kernel_spmd for NTFF profiling.
"""
import numpy as np
import ml_dtypes

import concourse.bass as bass
import concourse.bacc as bacc
import concourse.mybir as mybir
from concourse.tile import TileContext
from concourse.bass_utils import run_bass_kernel_spmd
from concourse.masks import make_identity

# ---- problem constants (hardcoded; kernel.py must be self-contained) ----
N = 262144
NCORES = 8
NLOC = N // NCORES          # 32768 points per core
P = 128
C = NLOC // P               # 256 columns
L = 16
F = 2
T = 1 << 19
M19 = T - 1
BASE, DESIRED = 16, 512
_SCALE = np.exp(np.log(DESIRED / BASE) / (L - 1))
RES = np.floor(BASE * _SCALE ** np.arange(L)).astype(np.float32)
PRIMES = (1, 2654435761, 805459861)
P1_19 = PRIMES[1] & M19
P2_19 = PRIMES[2] & M19
W = 32
SH_C0 = 0.28209479177387814
SH_C1 = 0.4886025119029199
SH_C2 = [1.0925484305920792, -1.0925484305920792, 0.31539156525252005,
         -1.0925484305920792, 0.5462742152960396]
SH_C3 = [-0.5900435899266435, 2.890611442640554, -0.4570457994644658,
         0.3731763325901154, -0.4570457994644658, 1.445305721320277,
         -0.5900435899266435]

f32 = mybir.dt.float32
i32 = mybir.dt.int32
bf16 = mybir.dt.bfloat16
Alu = mybir.AluOpType
Act = mybir.ActivationFunctionType

_NC_CACHE = {}
_LAST_RESULTS = None


def _bc(ap, n):
    """Broadcast an AP by appending a step-0 dim of size n."""
    return bass.AP(ap.tensor, ap.offset, list(ap.ap) + [[0, n]])


def _flat(ap):
    """Merge contiguous free dims of a [P, a, b, ...] AP into [P, a*b*...]."""
    dims = list(ap.ap)
    part, free = dims[0], dims[1:]
    n = 1
    for step, num in reversed(free):
        assert step == n, f"_flat: non-contiguous free dims {dims}"
        n *= num
    return bass.AP(ap.tensor, ap.offset, [part, [1, n]])


def _rows(ap, rows, elems):
    """View a contiguous [P, ...] region as [P, rows, elems].

    The HW indirect-DMA DGE emits one descriptor per out-AP row, so the
    out AP row structure MUST be one row per gather index (a flat out AP
    makes the HW stream sequential rows from the first index only).
    """
    dims = list(ap.ap)
    n = 1
    for step, num in reversed(dims[1:]):
        assert step == n, f"_rows: non-contiguous free dims {dims}"
        n *= num
    assert n == rows * elems, (n, rows, elems)
    return bass.AP(ap.tensor, ap.offset, [dims[0], [elems, rows], [1, elems]])


# levels 0..DENSE_L-1 use dense per-cell tables (one 64B gather per point
# instead of eight 8B gathers); the rest use the hashed table directly.
DENSE_L = 12
NIDX_MAX = 8192
DR1 = [int(RES[l]) + 1 for l in range(L)]            # cells per axis
DOFF = np.cumsum([0] + [DR1[l] ** 3 for l in range(DENSE_L)]).tolist()
DROWS = DOFF[DENSE_L] if DENSE_L else 0


def build_nc(cols=C, debug=False):
    nloc = P * cols
    nc = bacc.Bacc(None)
    # xs/ds are host-pretransposed to [P, 3, cols] (dim-major) so per-dim
    # chains can run as single merged [P, 3*cols] DVE ops.
    xs_d = nc.dram_tensor("xs", [P, 3 * cols], f32, kind="ExternalInput")
    ds_d = nc.dram_tensor("ds", [P, 3 * cols], f32, kind="ExternalInput")
    emb_d = nc.dram_tensor("emb", [L * T, 4], bf16, kind="ExternalInput")
    if DENSE_L:
        dense_d = nc.dram_tensor("dense", [DROWS, 32], bf16,
                                 kind="ExternalInput")
    # wq columns: [lhsT1(128) | lhsT2(128) | lhsT3(92 -> pad 96)]
    wq_d = nc.dram_tensor("wq", [P, 352], bf16, kind="ExternalInput")
    bq_d = nc.dram_tensor("bq", [P, 3], f32, kind="ExternalInput")
    out_d = nc.dram_tensor("outc", [nloc, 3], f32, kind="ExternalOutput")
    if debug:
        acc_d = nc.dram_tensor("acc_dbg", [nloc, 64], f32, kind="ExternalOutput")
        mo_d = nc.dram_tensor("mo_dbg", [nloc, 46], f32, kind="ExternalOutput")
        idx_d = nc.dram_tensor("idx_dbg", [L, nloc, 8], i32, kind="ExternalOutput")
        w_d = nc.dram_tensor("w_dbg", [L, nloc, 8], bf16, kind="ExternalOutput")

    def tt(o, a, b_, op):
        nc.vector.tensor_tensor(out=o, in0=a, in1=b_, op=op)

    def ts(o, a, s1, s2, op0, op1=None):
        if op1 is None:
            nc.vector.tensor_scalar(out=o, in0=a, scalar1=s1, scalar2=None,
                                    op0=op0)
        else:
            nc.vector.tensor_scalar(out=o, in0=a, scalar1=s1, scalar2=s2,
                                    op0=op0, op1=op1)

    def stt(o, a, s, b_, op0, op1):
        nc.vector.scalar_tensor_tensor(out=o, in0=a, scalar=s, in1=b_,
                                       op0=op0, op1=op1)

    with TileContext(nc) as tc:
        with tc.tile_pool(name="persist", bufs=1) as pp:
            identb = pp.tile([P, P], bf16)
            make_identity(nc, identb[:])
            xst = pp.tile([P, 3, cols], f32)
            dst = pp.tile([P, 3, cols], f32)
            nc.sync.dma_start(out=_flat(xst[:]), in_=xs_d[:])
            nc.sync.dma_start(out=_flat(dst[:]), in_=ds_d[:])
            wq = pp.tile([P, 352], bf16)
            bq = pp.tile([P, 3], f32)
            nc.sync.dma_start(out=wq[:], in_=wq_d[:])
            nc.sync.dma_start(out=bq[:], in_=bq_d[:])
            acc = pp.tile([P, cols, 64], bf16)
            # int constant tiles
            c_one = pp.tile([P, 1], i32, tag="c_one")
            c_m19 = pp.tile([P, 1], i32, tag="c_m19")
            c_511 = pp.tile([P, 1], i32, tag="c_511")
            c_10 = pp.tile([P, 1], i32, tag="c_10")
            c_p1 = pp.tile([P, 1], i32, tag="c_p1")
            c_p2 = pp.tile([P, 1], i32, tag="c_p2")
            nc.gpsimd.memset(c_one[:], 1)
            nc.gpsimd.memset(c_m19[:], M19)
            nc.gpsimd.memset(c_511[:], 511)
            nc.gpsimd.memset(c_10[:], 10)
            nc.gpsimd.memset(c_p1[:], P1_19)
            nc.gpsimd.memset(c_p2[:], P2_19)

            # ---------------- hash-encode phase ----------------
            with tc.tile_pool(name="lvl", bufs=2) as lp:
                for lvl in range(L):
                    res = float(RES[lvl])
                    dense = lvl < DENSE_L
                    h0 = [None] * 3
                    h1 = [None] * 3
                    # merged per-dim chain on [P, 3, cols] (one op, 3 dims)
                    posm = lp.tile([P, 3, cols], f32, tag="posm")
                    ts(posm[:], xst[:], res, 0.5 * res - 0.5, Alu.mult, Alu.add)
                    # robust floor(pos), pos = posm + 0.5: the f32->i32
                    # convert truncates in CoreSim but rounds on HW, so
                    # convert, then fix up by comparing against pos.
                    i0c = lp.tile([P, 3, cols], i32, tag="i0c")
                    nc.vector.tensor_copy(out=i0c[:], in_=posm[:])
                    f0c = lp.tile([P, 3, cols], f32, tag="f0c")
                    nc.vector.tensor_copy(out=f0c[:], in_=i0c[:])
                    tpos = lp.tile([P, 3, cols], f32, tag="tpos")
                    stt(tpos[:], posm[:], 0.5, f0c[:], Alu.add, Alu.subtract)
                    up = lp.tile([P, 3, cols], f32, tag="up")
                    ts(up[:], tpos[:], 1.0, None, Alu.is_ge)
                    dn = lp.tile([P, 3, cols], f32, tag="dn")
                    ts(dn[:], tpos[:], 0.0, None, Alu.is_lt)
                    adj = up
                    tt(adj[:], up[:], dn[:], Alu.subtract)
                    f03 = f0c
                    tt(f03[:], f0c[:], adj[:], Alu.add)
                    fr3 = tpos
                    tt(fr3[:], tpos[:], adj[:], Alu.subtract)
                    om3 = lp.tile([P, 3, cols], f32, tag="om3")
                    ts(om3[:], fr3[:], -1.0, 1.0, Alu.mult, Alu.add)
                    w1 = [fr3[:, d, :] for d in range(3)]
                    w0 = [om3[:, d, :] for d in range(3)]
                    f0s = [f03[:, d, :] for d in range(3)]
                    i03 = None
                    if not dense:
                        i03 = lp.tile([P, 3, cols], i32, tag="i03")
                        nc.vector.tensor_copy(out=i03[:], in_=f03[:])
                    for d in range(3 if not dense else 0):
                        if d == 0:
                            h0[d] = i03[:, 0, :]
                            hx1 = lp.tile([P, cols], i32, tag="hx1")
                            tt(hx1[:], i03[:, 0, :], _bc(c_one[:, 0:1], cols),
                               Alu.add)
                            h1[d] = hx1[:]
                        else:
                            pc = P1_19 if d == 1 else P2_19
                            cpt = c_p1 if d == 1 else c_p2
                            bhi = float(pc >> 10)
                            clo = float(pc & 1023)
                            yb = lp.tile([P, cols], f32, tag="yb")
                            ts(yb[:], f0s[d], bhi, None, Alu.mult)
                            yc = lp.tile([P, cols], f32, tag="yc")
                            ts(yc[:], f0s[d], clo, None, Alu.mult)
                            iyb = lp.tile([P, cols], i32, tag="iyb")
                            nc.vector.tensor_copy(out=iyb[:], in_=yb[:])
                            tt(iyb[:], iyb[:], _bc(c_511[:, 0:1], cols),
                               Alu.bitwise_and)
                            tt(iyb[:], iyb[:], _bc(c_10[:, 0:1], cols),
                               Alu.logical_shift_left)
                            iyc = lp.tile([P, cols], i32, tag="iyc")
                            nc.vector.tensor_copy(out=iyc[:], in_=yc[:])
                            hh0 = lp.tile([P, cols], i32, tag=f"hh0{d}")
                            tt(hh0[:], iyb[:], iyc[:], Alu.add)
                            tt(hh0[:], hh0[:], _bc(c_m19[:, 0:1], cols),
                               Alu.bitwise_and)
                            h0[d] = hh0[:]
                            hh1 = lp.tile([P, cols], i32, tag=f"hh1{d}")
                            tt(hh1[:], hh0[:], _bc(cpt[:, 0:1], cols), Alu.add)
                            tt(hh1[:], hh1[:], _bc(c_m19[:, 0:1], cols),
                               Alu.bitwise_and)
                            h1[d] = hh1[:]

                    # corner weights; corner k = (a<<2)|(b<<1)|cz
                    w8 = lp.tile([P, cols, 8], bf16, tag="w8")
                    wyz = []
                    for b in range(2):
                        for cz in range(2):
                            j = b * 2 + cz
                            t_w = lp.tile([P, cols], f32, tag=f"wyz{j}")
                            tt(t_w[:], (w1[1] if b else w0[1]),
                               (w1[2] if cz else w0[2]), Alu.mult)
                            wyz.append(t_w)
                    for a in range(2):
                        wx = w1[0] if a else w0[0]
                        for j in range(4):
                            tt(w8[:, :, a * 4 + j], wx, wyz[j][:], Alu.mult)

                    if dense:
                        # linear cell index into the dense table:
                        # lin = (f0z*r1 + f0y)*r1 + f0x, exact in f32
                        r1 = float(DR1[lvl])
                        inner = lp.tile([P, cols], f32, tag="inner")
                        stt(inner[:], f0s[2], r1, f0s[1], Alu.mult, Alu.add)
                        linf = lp.tile([P, cols], f32, tag="linf")
                        stt(linf[:], inner[:], r1, f0s[0], Alu.mult, Alu.add)
                        il = lp.tile([P, cols], i32, tag="il")
                        nc.vector.tensor_copy(out=il[:], in_=linf[:])
                    else:
                        idx8 = lp.tile([P, cols, 8], i32, tag="idx8")
                        hyz = []
                        for b in range(2):
                            for cz in range(2):
                                j = b * 2 + cz
                                t_h = lp.tile([P, cols], i32, tag=f"hyz{j}")
                                tt(t_h[:], (h1[1] if b else h0[1]),
                                   (h1[2] if cz else h0[2]),
                                   Alu.bitwise_xor)
                                hyz.append(t_h)
                        for a in range(2):
                            hx = h1[0] if a else h0[0]
                            for j in range(4):
                                tt(idx8[:, :, a * 4 + j], hx, hyz[j][:],
                                   Alu.bitwise_xor)

                    if debug:
                        if dense:
                            nc.sync.dma_start(
                                out=idx_d[lvl, :, 0:1].rearrange(
                                    "(p c) k -> p c k", p=P),
                                in_=_bc(il[:], 1))
                        else:
                            nc.sync.dma_start(
                                out=idx_d[lvl].rearrange("(p c) k -> p c k",
                                                         p=P),
                                in_=idx8[:])
                        nc.sync.dma_start(
                            out=w_d[lvl].rearrange("(p c) k -> p c k", p=P),
                            in_=w8[:])
                    accsl = acc[:, :, 4 * lvl:4 * lvl + 4]
                    # max indices per indirect call (HW-validated size)
                    ncall_d = max(1, (P * cols) // NIDX_MAX)
                    ncall_h = max(1, (P * cols * 8) // NIDX_MAX)
                    if dense:
                        # dense rows are [f, k] so the whole interp runs on
                        # inner-contiguous bf16 APs (DVE 2x perf mode).
                        featsD = lp.tile([P, cols, 4, 8], bf16, tag="feats")
                        for cc in range(cols):
                            nc.gpsimd.indirect_dma_start(
                                out=_flat(featsD[:, cc, :, :]),
                                out_offset=None,
                                in_=dense_d[:],
                                in_offset=bass.IndirectOffsetOnAxis(
                                    ap=il[:, cc:cc + 1], axis=0),
                                element_offset=DOFF[lvl] * 32,
                            )
                        w8bc = bass.AP(w8[:].tensor, w8[:].offset,
                                       [w8[:].ap[0], [8, cols], [0, 4], [1, 8]])
                        tt(featsD[:], featsD[:], w8bc, Alu.mult)
                        tt(featsD[:, :, :, 0:4], featsD[:, :, :, 0:4],
                           featsD[:, :, :, 4:8], Alu.add)
                        tt(featsD[:, :, :, 0:2], featsD[:, :, :, 0:2],
                           featsD[:, :, :, 2:4], Alu.add)
                        tt(accsl, featsD[:, :, :, 0], featsD[:, :, :, 1],
                           Alu.add)
                    else:
                        feats8 = lp.tile([P, cols, 32], bf16, tag="feats")
                        for cc in range(cols):
                            for k in range(8):
                                nc.gpsimd.indirect_dma_start(
                                    out=feats8[:, cc, 4 * k:4 * k + 4],
                                    out_offset=None,
                                    in_=emb_d[:],
                                    in_offset=bass.IndirectOffsetOnAxis(
                                        ap=idx8[:, cc, k:k + 1], axis=0),
                                    element_offset=lvl * T * 4,
                                )
                        tmp = lp.tile([P, cols, 4], bf16, tag="tmpi")
                        tt(accsl, feats8[:, :, 0:4], _bc(w8[:, :, 0], 4),
                           Alu.mult)
                        for k in range(1, 8):
                            tt(tmp[:], feats8[:, :, 4 * k:4 * k + 4],
                               _bc(w8[:, :, k], 4), Alu.mult)
                            tt(accsl, accsl, tmp[:], Alu.add)

            # ---------------- MLP phase (fused, block-diag x2 cols) ----------
            with tc.tile_pool(name="mlp", bufs=1) as mp, \
                 tc.tile_pool(name="blk", bufs=3) as bp, \
                 tc.tile_pool(name="pst", bufs=2, space="PSUM") as pst, \
                 tc.tile_pool(name="psm", bufs=1, space="PSUM") as psm:
                outs_pm = mp.tile([P, cols, 46], f32)
                NBLK = cols // 8  # 4 col-pairs -> 512 matmul columns per block
                for blk in range(NBLK):
                    xTb = bp.tile([P, 512], bf16, tag="xTb")
                    for s2 in range(4):
                        cp = blk * 8 + s2 * 2
                        ptin = pst.tile([P, P], bf16, tag="ptin")
                        nc.tensor.transpose(
                            out=ptin[:],
                            in_=acc[:, cp:cp + 2, :],
                            identity=identb[:])
                        nc.scalar.copy(xTb[:, s2 * P:(s2 + 1) * P], ptin[:])
                    ps1 = psm.tile([P, 512], f32, tag="ps1")
                    nc.tensor.matmul(ps1[:], lhsT=wq[:, 0:128], rhs=xTb[:],
                                     start=True, stop=True)
                    h1b = bp.tile([P, 512], bf16, tag="h1b")
                    nc.scalar.activation(h1b[:], ps1[:], Act.Relu,
                                         bias=bq[:, 0:1])
                    ps2 = psm.tile([P, 512], f32, tag="ps2")
                    nc.tensor.matmul(ps2[:], lhsT=wq[:, 128:256], rhs=h1b[:],
                                     start=True, stop=True)
                    h2b = bp.tile([P, 512], bf16, tag="h2b")
                    nc.scalar.activation(h2b[:], ps2[:], Act.Relu,
                                         bias=bq[:, 1:2])
                    ps3 = psm.tile([92, 512], f32, tag="ps3")
                    nc.tensor.matmul(ps3[:], lhsT=wq[:, 256:348], rhs=h2b[:],
                                     start=True, stop=True)
                    o3b = bp.tile([92, 512], bf16, tag="o3b")
                    nc.scalar.activation(o3b[:], ps3[:], Act.Identity,
                                         bias=bq[:92, 2:3])
                    for s2 in range(4):
                        cp = blk * 8 + s2 * 2
                        ptout = pst.tile([P, 92], bf16, tag="ptout")
                        nc.tensor.transpose(
                            out=ptout[:],
                            in_=o3b[:, s2 * P:(s2 + 1) * P],
                            identity=identb[:92, :92])
                        nc.scalar.copy(outs_pm[:, cp, :], ptout[:, 0:46])
                        nc.scalar.copy(outs_pm[:, cp + 1, :],
                                       ptout[:, 46:92])

                if debug:
                    accf = mp.tile([P, cols, 64], f32, tag="accf")
                    nc.vector.tensor_copy(out=accf[:], in_=acc[:])
                    nc.sync.dma_start(
                        out=acc_d[:].rearrange("(p c) d -> p c d", p=P),
                        in_=accf[:])
                    nc.sync.dma_start(
                        out=mo_d[:].rearrange("(p c) d -> p c d", p=P),
                        in_=outs_pm[:])

                # ---- SH eval + final tiny matmuls (points-major, wide) ----
                sh = outs_pm  # [:, :, 0:16] = sh coeffs, [:, :, 16:46] = ws
                tA = mp.tile([P, cols], f32, tag="tA")
                tB = mp.tile([P, cols], f32, tag="tB")
                dx = mp.tile([P, cols], f32, tag="dx")
                dy = mp.tile([P, cols], f32, tag="dy")
                dz = mp.tile([P, cols], f32, tag="dz")
                r2 = mp.tile([P, cols], f32, tag="r2")
                tt(r2[:], dst[:, 0, :], dst[:, 0, :], Alu.mult)
                tt(tA[:], dst[:, 1, :], dst[:, 1, :], Alu.mult)
                tt(r2[:], r2[:], tA[:], Alu.add)
                tt(tA[:], dst[:, 2, :], dst[:, 2, :], Alu.mult)
                tt(r2[:], r2[:], tA[:], Alu.add)
                inv = mp.tile([P, cols], f32, tag="inv")
                nc.vector.reciprocal(out=inv[:], in_=r2[:])
                sc = mp.tile([P, cols], f32, tag="sc")
                nc.scalar.activation(sc[:], inv[:], Act.Sqrt)
                tt(dx[:], dst[:, 0, :], sc[:], Alu.mult)
                tt(dy[:], dst[:, 1, :], sc[:], Alu.mult)
                tt(dz[:], dst[:, 2, :], sc[:], Alu.mult)

                xx = mp.tile([P, cols], f32, tag="xx")
                yy = mp.tile([P, cols], f32, tag="yy")
                zz = mp.tile([P, cols], f32, tag="zz")
                xy = mp.tile([P, cols], f32, tag="xy")
                yz = mp.tile([P, cols], f32, tag="yz")
                xz = mp.tile([P, cols], f32, tag="xz")
                tt(xx[:], dx[:], dx[:], Alu.mult)
                tt(yy[:], dy[:], dy[:], Alu.mult)
                tt(zz[:], dz[:], dz[:], Alu.mult)
                tt(xy[:], dx[:], dy[:], Alu.mult)
                tt(yz[:], dy[:], dz[:], Alu.mult)
                tt(xz[:], dx[:], dz[:], Alu.mult)

                cres = mp.tile([P, cols], f32, tag="cres")

                def addterm(basis, k, coef):
                    """cres += coef * basis * sh[..k]; basis AP or None=1."""
                    if basis is None:
                        ts(tB[:], sh[:, :, k], coef, None, Alu.mult)
                    else:
                        stt(tB[:], sh[:, :, k], coef, basis, Alu.mult, Alu.mult)
                    tt(cres[:], cres[:], tB[:], Alu.add)

                ts(cres[:], sh[:, :, 0], SH_C0, None, Alu.mult)
                addterm(dy[:], 1, -SH_C1)
                addterm(dz[:], 2, SH_C1)
                addterm(dx[:], 3, -SH_C1)
                addterm(xy[:], 4, SH_C2[0])
                addterm(yz[:], 5, SH_C2[1])
                # C2[2]*(2zz-xx-yy)
                ts(tA[:], zz[:], 2.0, None, Alu.mult)
                tt(tA[:], tA[:], xx[:], Alu.subtract)
                tt(tA[:], tA[:], yy[:], Alu.subtract)
                addterm(tA[:], 6, SH_C2[2])
                addterm(xz[:], 7, SH_C2[3])
                xmy = mp.tile([P, cols], f32, tag="xmy")
                tt(xmy[:], xx[:], yy[:], Alu.subtract)
                addterm(xmy[:], 8, SH_C2[4])
                # C3 terms
                ts(tA[:], xx[:], 3.0, None, Alu.mult)
                tt(tA[:], tA[:], yy[:], Alu.subtract)
                tt(tA[:], tA[:], dy[:], Alu.mult)
                addterm(tA[:], 9, SH_C3[0])
                tt(tA[:], xy[:], dz[:], Alu.mult)
                addterm(tA[:], 10, SH_C3[1])
                ts(tA[:], zz[:], 4.0, None, Alu.mult)
                tt(tA[:], tA[:], xx[:], Alu.subtract)
                tt(tA[:], tA[:], yy[:], Alu.subtract)
                ttmp = mp.tile([P, cols], f32, tag="ttmp")
                nc.vector.tensor_copy(out=ttmp[:], in_=tA[:])
                tt(tA[:], tA[:], dy[:], Alu.mult)
                addterm(tA[:], 11, SH_C3[2])
                # C3[3]*z*(2zz-3xx-3yy)
                ts(tA[:], zz[:], 2.0, None, Alu.mult)
                ts(tB[:], xx[:], 3.0, None, Alu.mult)
                tt(tA[:], tA[:], tB[:], Alu.subtract)
                ts(tB[:], yy[:], 3.0, None, Alu.mult)
                tt(tA[:], tA[:], tB[:], Alu.subtract)
                tt(tA[:], tA[:], dz[:], Alu.mult)
                addterm(tA[:], 12, SH_C3[3])
                tt(tA[:], ttmp[:], dx[:], Alu.mult)
                addterm(tA[:], 13, SH_C3[4])
                tt(tA[:], xmy[:], dz[:], Alu.mult)
                addterm(tA[:], 14, SH_C3[5])
                tt(tA[:], xmy[:], dx[:], Alu.mult)
                addterm(tA[:], 15, SH_C3[6])

                # final: c1_j = relu(cres*m1_j + b1_j)  (m1=ws[0:3], b1=ws[3:6])
                ws0 = 16
                c1 = [mp.tile([P, cols], f32, name=f"c1_{j}", tag=f"c1_{j}") for j in range(3)]
                for j in range(3):
                    tt(c1[j][:], cres[:], sh[:, :, ws0 + j], Alu.mult)
                    tt(c1[j][:], c1[j][:], sh[:, :, ws0 + 3 + j], Alu.add)
                    ts(c1[j][:], c1[j][:], 0.0, None, Alu.max)
                c2 = [mp.tile([P, cols], f32, name=f"c2_{j}", tag=f"c2_{j}") for j in range(3)]
                for j in range(3):
                    tt(c2[j][:], c1[0][:], sh[:, :, ws0 + 6 + j], Alu.mult)
                    for s in range(1, 3):
                        tt(tB[:], c1[s][:], sh[:, :, ws0 + 6 + s * 3 + j],
                           Alu.mult)
                        tt(c2[j][:], c2[j][:], tB[:], Alu.add)
                    tt(c2[j][:], c2[j][:], sh[:, :, ws0 + 15 + j], Alu.add)
                    ts(c2[j][:], c2[j][:], 0.0, None, Alu.max)
                outt = mp.tile([P, cols, 3], f32, tag="outt")
                for j in range(3):
                    tt(tA[:], c2[0][:], sh[:, :, ws0 + 18 + j], Alu.mult)
                    for s in range(1, 3):
                        tt(tB[:], c2[s][:], sh[:, :, ws0 + 18 + s * 3 + j],
                           Alu.mult)
                        tt(tA[:], tA[:], tB[:], Alu.add)
                    tt(tA[:], tA[:], sh[:, :, ws0 + 27 + j], Alu.add)
                    nc.scalar.activation(outt[:, :, j], tA[:], Act.Sigmoid)

                nc.sync.dma_start(out=out_d[:].rearrange("(p c) d -> p c d", p=P),
                                  in_=outt[:])
    nc.compile()
    return nc


def prep_dense(emb_il):
    """Dense per-cell corner tables for levels < DENSE_L.

    Row x + r1*y + r1^2*z of level lvl holds the 8 hashed corner feature
    rows of cell (x, y, z), in corner order k = (dx<<2)|(dy<<1)|dz.
    """
    if not DENSE_L:
        return np.zeros((0, 32), ml_dtypes.bfloat16)
    pieces = []
    for lvl in range(DENSE_L):
        r1 = DR1[lvl]
        g = np.arange(r1 + 1, dtype=np.uint32)
        hx = g * np.uint32(PRIMES[0])
        hy = g * np.uint32(PRIMES[1])
        hz = g * np.uint32(PRIMES[2])
        out = np.empty((r1 ** 3, 4, 8), ml_dtypes.bfloat16)  # row = [f, k]
        base = lvl * T
        for k in range(8):
            a, b, c = (k >> 2) & 1, (k >> 1) & 1, k & 1
            h = ((hz[c:c + r1][:, None, None]
                  ^ hy[b:b + r1][None, :, None]
                  ^ hx[a:a + r1][None, None, :]) & np.uint32(M19))
            out[:, :, k] = emb_il[base + h.ravel().astype(np.int64)]
        pieces.append(out.reshape(r1 ** 3, 32))
    return np.concatenate(pieces, axis=0)


def prep_tables(emb_x, emb_w, lw1, lb1, lw2, lb2, lw3, lb3,
                ww1, wb1, ww2, wb2, ww3, wb3):
    emb_il = np.concatenate(
        [np.asarray(emb_x, np.float32).reshape(L * T, F),
         np.asarray(emb_w, np.float32).reshape(L * T, F)],
        axis=1).astype(ml_dtypes.bfloat16)  # [L*T, 4]

    W1c = np.zeros((64, 64), np.float32)
    for lvl in range(L):
        W1c[4 * lvl + 0, 0:32] = lw1[2 * lvl]
        W1c[4 * lvl + 1, 0:32] = lw1[2 * lvl + 1]
        W1c[4 * lvl + 2, 32:64] = ww1[2 * lvl]
        W1c[4 * lvl + 3, 32:64] = ww1[2 * lvl + 1]
    W2c = np.zeros((64, 64), np.float32)
    W2c[0:32, 0:32] = lw2
    W2c[32:64, 32:64] = ww2
    W3c = np.zeros((64, 46), np.float32)
    W3c[0:32, 0:16] = lw3
    W3c[32:64, 16:46] = ww3

    wq = np.zeros((P, 352), np.float32)
    wq[0:64, 0:64] = W1c
    wq[64:128, 64:128] = W1c
    wq[0:64, 128:192] = W2c
    wq[64:128, 192:256] = W2c
    wq[0:64, 256:302] = W3c
    wq[64:128, 302:348] = W3c
    wq = wq.astype(ml_dtypes.bfloat16)

    b1c = np.concatenate([lb1, wb1])                  # [64]
    b2c = np.concatenate([lb2, wb2])                  # [64]
    b3c = np.concatenate([lb3, wb3])                  # [46]
    bq = np.zeros((P, 3), np.float32)
    bq[:, 0] = np.concatenate([b1c, b1c])
    bq[:, 1] = np.concatenate([b2c, b2c])
    bq[:92, 2] = np.concatenate([b3c, b3c])
    return emb_il, wq, bq


def _fingerprint(*arrays):
    parts = []
    for a in arrays:
        a = np.asarray(a)
        flat = a.reshape(-1)
        parts.append((a.shape, str(a.dtype), flat[:16].tobytes(),
                      flat[-16:].tobytes(), flat[::max(1, flat.size // 64)]
                      .tobytes()))
    return hash(str(parts))


def _make_runner(nc):
    import jax
    from jax.sharding import Mesh, PartitionSpec
    from jax.experimental.shard_map import shard_map
    from concourse import bass2jax
    from concourse.bass2jax import _bass_exec_p, install_neuronx_cc_hook

    install_neuronx_cc_hook()
    assert not nc.dbg_callbacks
    partition_name = (nc.partition_id_tensor.name
                      if nc.partition_id_tensor else None)
    dbg_name = nc.dbg_addr.name if nc.dbg_addr is not None else None

    in_names, out_names, out_avals, zero_shapes = [], [], [], []
    in_shapes = {}
    for alloc in nc.m.functions[0].allocations:
        if not isinstance(alloc, mybir.MemoryLocationSet):
            continue
        name = alloc.memorylocations[0].name
        if alloc.kind == "ExternalInput":
            if name == partition_name:
                continue
            in_names.append(name)
            if alloc.tensor_shape is not None:
                in_shapes[name] = tuple(alloc.tensor_shape)
        elif alloc.kind == "ExternalOutput":
            out_names.append(name)
            shape = tuple(alloc.tensor_shape)
            dtype = mybir.dt.np(alloc.dtype)
            out_avals.append(jax.core.ShapedArray(shape, dtype))
            zero_shapes.append((shape, dtype))
    n_params = len(in_names)
    n_outs = len(out_names)
    all_names = in_names + out_names
    donate = tuple(range(n_params, n_params + n_outs))

    def _body(*args):
        operands = list(args)
        if partition_name is not None:
            operands.append(bass2jax.partition_id_tensor())
        outs = _bass_exec_p.bind(
            *operands,
            out_avals=tuple(out_avals),
            in_names=tuple(all_names
                           + ([partition_name] if partition_name else [])),
            out_names=tuple(out_names),
            lowering_input_output_aliases=(),
            sim_require_finite=True,
            sim_require_nnan=True,
            nc=nc,
        )
        return tuple(outs)

    devices = jax.devices()[:NCORES]
    mesh = Mesh(np.asarray(devices), ("core",))
    spec = jax.sharding.NamedSharding(mesh, PartitionSpec("core"))
    jitted = jax.jit(
        shard_map(_body, mesh=mesh,
                  in_specs=(PartitionSpec("core"),) * (n_params + n_outs),
                  out_specs=(PartitionSpec("core"),) * n_outs,
                  check_rep=False),
        donate_argnums=donate, keep_unused=True)

    def put_replicated(arr):
        import jax as _jax
        gshape = (NCORES * arr.shape[0],) + arr.shape[1:]
        return _jax.make_array_from_callback(gshape, spec, lambda idx: arr)

    return {"jitted": jitted, "in_names": in_names, "out_names": out_names,
            "zero_shapes": zero_shapes, "spec": spec, "dbg_name": dbg_name,
            "in_shapes": in_shapes, "put_replicated": put_replicated}


def _dim_major(a):
    """[N, 3] -> per-core [P, 3, C] layout, stacked: [NCORES*P, 3*C]."""
    return np.ascontiguousarray(
        a.reshape(NCORES, P, C, 3).transpose(0, 1, 3, 2)
    ).reshape(NCORES * P, 3 * C)


def kernel(xs, ds, emb_x, emb_w, lw1, lb1, lw2, lb2, lw3, lb3,
           ww1, wb1, ww2, wb2, ww3, wb3):
    global _LAST_RESULTS
    import os
    xs = _dim_major(np.asarray(xs, dtype=np.float32))
    ds = _dim_major(np.asarray(ds, dtype=np.float32))

    fp = _fingerprint(emb_x, emb_w, lw1, lw2, lw3, ww1, ww2, ww3,
                      lb1, lb2, lb3, wb1, wb2, wb3)
    if _NC_CACHE.get("const_fp") != fp:
        emb_il, wq, bq = prep_tables(emb_x, emb_w, lw1, lb1, lw2, lb2,
                                     lw3, lb3, ww1, wb1, ww2, wb2, ww3, wb3)
        dense = prep_dense(emb_il)
        _NC_CACHE["consts"] = {"emb": emb_il, "wq": wq, "bq": bq}
        if DENSE_L:
            _NC_CACHE["consts"]["dense"] = dense
        _NC_CACHE["const_fp"] = fp
        _NC_CACHE.pop("dev_consts", None)
    consts = _NC_CACHE["consts"]

    if "nc" not in _NC_CACHE:
        _NC_CACHE["nc"] = build_nc()
    nc = _NC_CACHE["nc"]

    if os.environ.get("BASS_TRACE"):
        # slow traced path (ships all tables every call, captures NTFF)
        in_maps = []
        for r in range(NCORES):
            sl = slice(r * P, (r + 1) * P)
            im = {"xs": np.ascontiguousarray(xs[sl]),
                  "ds": np.ascontiguousarray(ds[sl])}
            im.update(consts)
            in_maps.append(im)
        res = run_bass_kernel_spmd(nc, in_maps, list(range(NCORES)))
        _LAST_RESULTS = res
        return np.concatenate(
            [res.results[r]["outc"] for r in range(NCORES)], axis=0)

    if "runner" not in _NC_CACHE:
        _NC_CACHE["runner"] = _make_runner(nc)
    r = _NC_CACHE["runner"]
    if "dev_consts" not in _NC_CACHE:
        _NC_CACHE["dev_consts"] = {k: r["put_replicated"](v)
                                   for k, v in consts.items()}
    dev_consts = _NC_CACHE["dev_consts"]

    args = []
    for name in r["in_names"]:
        if name == "xs":
            args.append(xs)
        elif name == "ds":
            args.append(ds)
        elif name == r["dbg_name"]:
            sh = r["in_shapes"][name]
            args.append(np.zeros((NCORES * sh[0],) + tuple(sh[1:]),
                                 np.uint32))
        else:
            args.append(dev_consts[name])
    zeros = [np.zeros((NCORES * s[0],) + tuple(s[1:]), d)
             for s, d in r["zero_shapes"]]
    outs = r["jitted"](*args, *zeros)
    out = np.asarray(outs[r["out_names"].index("outc")])
    _LAST_RESULTS = None
    return out
